# revision 6
# baseline (speedup 1.0000x reference)
"""Sigmoid-attention MHA kernel for 8 Trainium2 NeuronCores (v3).

Problem: x[4,2048,512], W_q/W_k/W_v/W_o[512,512] (already scaled).
  Q = x@Wq.T, K = x@Wk.T, V = x@Wv.T split into 8 heads of depth 64
  attn = sigmoid(QK^T/sqrt(64) - log(2048));  out = (attn@V merged)@Wo.T

Sharding: core c handles batch b=c//2, head-group g=c%2 (4 heads each).
Each core computes a partial output projection over its 256 head-features;
host sums the two partials per batch.

v3: the kernel is ScalarE(ACT)-bound: 16.8M sigmoid elements/core at
1 elem/lane/cycle @1.2GHz with a 352-cycle pipeline fill per ACTIVATE.
v2 ran 128 ACTIVATEs of N=1024 (~1147ns each -> 156us E2E). v3 batches
scores into two 3-bank PSUM slots (A/B, [128,1536] each, 3 units of
512 queries) and runs 86 ACTIVATEs of N=1536 (~1573ns) -> ~139us ACT.
The 2 extra PSUM banks come from V-projection, which moved to a
serialized prologue on the opsum bank (before attn@V's accumulation
group starts); UDELAY covers it with a-tile backlog in SBUF.

Engine plan (per core):
  PE      scores (f32r, 1 unit = 512 q cols/mm), attn@V (a-stationary,
          V-moving bf16), O-block transposes, Q/K/V/Wo projections.
  ScalarE ONLY sigmoid ACTIVATEs (PSUM->SBUF bf16, N=1536).
  DVE     all PSUM->SBUF copies (proj, V, obf staging, psT->ot, waves).
  SP      all input DMAs + output wave DMAs (plus scalar/pool-queue DMAs
          in the prologue where those queues are idle anyway).
"""

import os
import numpy as np

DEBUG = bool(int(os.environ.get("KERNEL_DEBUG", "0")))
LOOP = int(os.environ.get("KERNEL_LOOP", "0"))  # >0: wrap body in For_i (timing)
UDELAY = int(os.environ.get("KERNEL_UDELAY", "36"))  # attnV emission delay (units)
SRESET = bool(int(os.environ.get("KERNEL_SRESET", "1")))

B, S, D = 4, 2048, 512
NH, DEPTH = 8, 64
G = 2          # head groups (one per core pair)
GF = 256       # features per group
NEG_LOG_S = float(np.float32(-np.log(np.float32(S))))
INV_SQRT_DK = 0.125

_CACHE = {}


def _build_nc():
    import concourse.bacc as bacc
    import concourse.tile as tile
    from concourse import mybir

    f32 = mybir.dt.float32
    f32r = mybir.dt.float32r
    bf16 = mybir.dt.bfloat16
    nc = bacc.Bacc("TRN2", target_bir_lowering=False, debug=False, num_devices=8)

    xt_d = nc.dram_tensor("xt", [128, 8192], f32r, kind="ExternalInput").ap()
    wq_d = nc.dram_tensor("wq", [128, 1024], f32r, kind="ExternalInput").ap()
    wk_d = nc.dram_tensor("wk", [128, 1024], f32r, kind="ExternalInput").ap()
    wv_d = nc.dram_tensor("wv", [128, 1024], f32r, kind="ExternalInput").ap()
    wo_d = nc.dram_tensor("wo", [128, 1024], f32, kind="ExternalInput").ap()
    id_d = nc.dram_tensor("ident", [128, 128], f32, kind="ExternalInput").ap()
    out_d = nc.dram_tensor("out", [S, D], f32, kind="ExternalOutput").ap()

    with tile.TileContext(nc) as tc:
        with (
            tc.tile_pool(name="persist", bufs=1) as persist,
            tc.tile_pool(name="attn", bufs=15) as apool,
            tc.tile_pool(name="stage", bufs=3) as stage,
            tc.tile_pool(name="apsum", bufs=1, space="PSUM") as apsum,
            tc.tile_pool(name="bpsum", bufs=1, space="PSUM") as bpsum,
            tc.tile_pool(name="opsum", bufs=1, space="PSUM") as opsum,
            tc.tile_pool(name="mpsum", bufs=1, space="PSUM") as mpsum,
        ):
            import contextlib
            if LOOP > 0:
                loop_cm = tc.For_i(0, LOOP, 1, staggered_reset=SRESET)
            else:
                loop_cm = contextlib.nullcontext()
            Sig = mybir.ActivationFunctionType.Sigmoid

            def mm(out, lhsT, rhs, start, stop):
                # f32r: single-pass fp32 matmul (4x faster than fp32 on PE)
                nc.tensor.matmul(out, lhsT=lhsT.bitcast(f32r),
                                 rhs=rhs.bitcast(f32r), start=start, stop=stop)

            def mmb(out, lhsT, rhs, start, stop):
                nc.tensor.matmul(out, lhsT=lhsT, rhs=rhs, start=start,
                                 stop=stop, skip_group_check=True)

            with loop_cm:
                bias_t = persist.tile([128, 1], f32, tag="bias", name="bias_t")
                nc.vector.memset(bias_t[:], NEG_LOG_S)
                warm_t = persist.tile([128, 1], f32, tag="warm", name="warm_t")
                nc.scalar.activation(warm_t[:], bias_t[:], Sig, bias=bias_t[:])

                wq_sb = persist.tile([128, 1024], f32r, tag="wq", name="wq_sb")
                wk_sb = persist.tile([128, 1024], f32r, tag="wk", name="wk_sb")
                wv_sb = persist.tile([128, 1024], f32r, tag="wv", name="wv_sb")
                wo_f = persist.tile([128, 1024], f32, tag="wof", name="wo_f")
                wo_sb = persist.tile([128, 1024], bf16, tag="wo", name="wo_sb")
                id_f = persist.tile([128, 128], f32, tag="idf", name="id_f")
                idb = persist.tile([128, 128], bf16, tag="idb", name="idb")
                xt = [persist.tile([128, 2048], f32r, tag=f"xt{c}", name=f"xt{c}")
                      for c in range(4)]
                nc.sync.dma_start(out=xt[0][:], in_=xt_d[:, 0:2048])
                nc.scalar.dma_start(out=xt[1][:], in_=xt_d[:, 2048:4096])
                nc.gpsimd.dma_start(out=xt[2][:], in_=xt_d[:, 4096:6144])
                nc.sync.dma_start(out=xt[3][:], in_=xt_d[:, 6144:8192])
                nc.scalar.dma_start(out=wq_sb[:], in_=wq_d[:])
                nc.gpsimd.dma_start(out=wk_sb[:], in_=wk_d[:])
                nc.sync.dma_start(out=wv_sb[:], in_=wv_d[:])
                nc.scalar.dma_start(out=wo_f[:], in_=wo_d[:])
                nc.gpsimd.dma_start(out=id_f[:], in_=id_d[:])
                nc.vector.tensor_copy(wo_sb[:], wo_f[:])
                nc.vector.tensor_copy(idb[:], id_f[:])

                qt = [persist.tile([128, 2048], f32r, tag=f"qt{m}", name=f"qt{m}")
                      for m in range(2)]
                kt = [persist.tile([128, 2048], f32r, tag=f"kt{m}", name=f"kt{m}")
                      for m in range(2)]
                v = [persist.tile([128, 512], bf16, tag=f"v{t}", name=f"v{t}")
                     for t in range(8)]
                ot = [persist.tile([128, 2048], bf16, tag=f"ot{m}", name=f"ot{m}")
                      for m in range(2)]

                # ---- Q/K projection chains (emitted just-in-time) ----
                def proj_chain(which, mc, qc):
                    w_sb = (wq_sb, wk_sb)[which]
                    dst = (qt, kt)[which][mc]
                    ps = mpsum.tile([128, 512], f32, tag="m", name="psP")
                    for kc4 in range(4):
                        mm(ps[:, 0:512],
                           w_sb[:, 256 * kc4 + 128 * mc:256 * kc4 + 128 * mc + 128],
                           xt[kc4][:, 512 * qc:512 * (qc + 1)],
                           start=(kc4 == 0), stop=(kc4 == 3))
                    nc.vector.tensor_copy(
                        dst[:, 512 * qc:512 * (qc + 1)], ps[:, 0:512])

                # ---- V projection pairs on the opsum bank (prologue) ----
                def emit_v_pair(t2):
                    pv = opsum.tile([128, 512], f32, tag="o", name="psV")
                    for half in range(2):
                        tck = 2 * t2 + half
                        cs = slice(256 * half, 256 * (half + 1))
                        for vkc in range(4):
                            mm(pv[:, cs],
                               xt[vkc][:, 128 * tck:128 * (tck + 1)],
                               wv_sb[:, 256 * vkc:256 * (vkc + 1)],
                               start=(vkc == 0 and half == 0),
                               stop=(vkc == 3 and half == 1))
                    nc.vector.tensor_copy(v[t2][:], pv[:, 0:512])

                # ---- units ----
                # unit u: tile t=u//2 (qc=t//32, p=(t//16)%2, kc=t%16), h=u%2
                # slot s=u//3 (A if s even else B), column offset 512*(u%3)
                NU = 256  # units (2 per tile: 128 tiles x 2 head-halves)
                def uinfo(u):
                    t = u // 2
                    return t // 32, (t // 16) % 2, t % 16, u % 2

                def emit_scores_unit(u, slot_ps):
                    qc, p, kc, h = uinfo(u)
                    ks = slice(128 * kc, 128 * (kc + 1))
                    qs = slice(512 * qc, 512 * (qc + 1))
                    off = 512 * (u % 3)
                    mm(slot_ps[:, off:off + 512], kt[p][64 * h:64 * h + 64, ks],
                       qt[p][64 * h:64 * h + 64, qs], start=True, stop=True)

                def emit_sigmoid(slot_ps, n):
                    a = apool.tile([128, 1536], bf16, tag="a", name="attn")
                    nc.scalar.activation(a[:, 0:512 * n], slot_ps[:, 0:512 * n],
                                         Sig, bias=bias_t[:], scale=INV_SQRT_DK)
                    return a

                psO = [None]

                def attn_v_unit(u, a, off):
                    # 4 moving-V matmuls for unit u; returns obf at group end
                    qc, p, kc, h = uinfo(u)
                    if kc == 0 and h == 0:
                        psO[0] = opsum.tile([128, 512], f32, tag="o", name="psO")
                    for j in range(4):
                        mmb(psO[0][:, 128 * j + 64 * h:128 * j + 64 * h + 64],
                            a[:, off + 128 * j:off + 128 * j + 128],
                            v[kc // 2][:, 256 * (kc % 2) + 128 * p + 64 * h:
                                        256 * (kc % 2) + 128 * p + 64 * h + 64],
                            start=(kc == 0 and j == 0 and h == 0),
                            stop=(kc == 15 and j == 3 and h == 1))
                    if kc == 15 and h == 1:
                        obf = stage.tile([128, 512], bf16, tag="obf", name="obf")
                        nc.vector.tensor_copy(obf[:], psO[0][:])
                        return obf
                    return None

                def emit_transposes(obf):
                    psT = mpsum.tile([128, 512], f32, tag="m",
                                     name="psT")[:].bitcast(bf16)
                    for j in range(4):
                        nc.tensor.matmul(psT[:, 128 * j:128 * (j + 1)],
                                         lhsT=obf[:, 128 * j:128 * (j + 1)],
                                         rhs=idb[:], is_transpose=True,
                                         start=(j == 0), stop=(j == 3),
                                         skip_group_check=True)
                    return psT

                def emit_ot_copy(p, qc, psT):
                    qs = slice(512 * qc, 512 * (qc + 1))
                    nc.vector.tensor_copy(ot[p][:, qs], psT[:, 0:512])

                def emit_wave(qc, w):
                    # tokens 512qc+128w : out rows <- ot[:, block] @ wo
                    tk = slice(512 * qc + 128 * w, 512 * qc + 128 * (w + 1))
                    psW = mpsum.tile([128, 512], f32, tag="m", name="psW")
                    for c in range(2):
                        mmb(psW[:], ot[c][:, tk], wo_sb[:, 512 * c:512 * (c + 1)],
                            start=(c == 0), stop=(c == 1))
                    st = stage.tile([128, 512], f32, tag="st", name="st")
                    nc.vector.tensor_copy(st[:], psW[:])
                    nc.sync.dma_start(out=out_d[tk, :], in_=st[:])

                # ---- schedule ----
                # proj chain deadlines in unit index (chain needed by unit dl)
                proj_chain(0, 0, 0)   # qt0 qc0
                proj_chain(1, 0, 0)   # kt0 qc0
                chain_deadlines = (
                    [(1, 0, qcK, 8 * qcK) for qcK in range(1, 4)] +
                    [(0, 1, 0, 32)] +
                    [(1, 1, qcK, 32 + 8 * qcK) for qcK in range(4)] +
                    [(0, 0, 1, 64), (0, 1, 1, 96), (0, 0, 2, 128),
                     (0, 1, 2, 160), (0, 0, 3, 192), (0, 1, 3, 224)])
                sched = {}
                for which, mc, qcK, dl in chain_deadlines:
                    sched.setdefault(max(0, dl - 8), []).append(
                        (lambda w=which, m=mc, q=qcK: proj_chain(w, m, q)))

                # V pairs: all 8 on opsum, interleaved with the first units
                for u0 in range(8):
                    sched.setdefault(2 * u0, []).append(
                        (lambda t2=u0: emit_v_pair(t2)))

                attn_q = {}     # unit -> (a_tile, col offset)
                obf_pend = {}
                ot_pend = {}
                wave_q = []
                slot_ps = [None]
                for i in range(NU + UDELAY + 16):
                    # 1. attnV for the delayed unit (PE) + group-end obf (DVE)
                    j = i - UDELAY
                    if UDELAY <= i < NU + UDELAY:
                        a, off = attn_q.pop(j)
                        obf = attn_v_unit(j, a, off)
                        if obf is not None:
                            qc, p, kc, h = uinfo(j)
                            obf_pend[i + 2] = (p, qc, obf)
                            if p == 1:
                                wave_q.extend((i + 6 + 4 * w, qc, w)
                                              for w in range(4))
                    # 2. new scores unit + sigmoid per 3 units
                    if i < NU:
                        if i % 3 == 0:
                            pool = apsum if (i // 3) % 2 == 0 else bpsum
                            slot_ps[0] = pool.tile(
                                [128, 1536], f32,
                                tag="ab"[(i // 3) % 2], name="psS")
                        emit_scores_unit(i, slot_ps[0])
                        if i % 3 == 2 or i == NU - 1:
                            n = i % 3 + 1
                            a = emit_sigmoid(slot_ps[0], n)
                            for k in range(n):
                                attn_q[i - n + 1 + k] = (a, 512 * k)
                    # 3. group-end pipeline: transposes, then ot copy
                    if i in obf_pend:
                        p2, q2, o2 = obf_pend.pop(i)
                        ot_pend[i + 2] = (p2, q2, emit_transposes(o2))
                    if i in ot_pend:
                        emit_ot_copy(*ot_pend.pop(i))
                    # 4. scheduled proj chains + V pairs
                    for fn in sched.pop(i, []):
                        fn()
                    # 5. output waves
                    if wave_q and i >= wave_q[0][0]:
                        emit_wave(*wave_q.pop(0)[1:])
                while wave_q:
                    emit_wave(*wave_q.pop(0)[1:])

    nc.compile()
    return nc


def get_nc():
    if "nc" not in _CACHE:
        _CACHE["nc"] = _build_nc()
    return _CACHE["nc"]


def make_in_maps(x, W_q, W_k, W_v, W_o):
    x = np.ascontiguousarray(np.asarray(x, dtype=np.float32))
    ws = [np.asarray(w, dtype=np.float32) for w in (W_q, W_k, W_v, W_o)]
    W_q, W_k, W_v, W_o = ws

    def chunked(a, nchunks):
        # [128*nchunks, m] -> [128, nchunks*m] with chunk-major columns
        m = a.shape[1]
        return np.ascontiguousarray(
            a.reshape(nchunks, 128, m).transpose(1, 0, 2).reshape(128, nchunks * m))

    ident = np.ascontiguousarray(np.eye(128, dtype=np.float32))
    in_maps = []
    for c in range(8):
        b, g = divmod(c, 2)
        gf = slice(GF * g, GF * (g + 1))
        in_maps.append({
            "xt": chunked(np.ascontiguousarray(x[b].T), 4),
            "wq": chunked(np.ascontiguousarray(W_q[gf, :].T), 4),
            "wk": chunked(np.ascontiguousarray(W_k[gf, :].T), 4),
            "wv": chunked(np.ascontiguousarray(W_v[gf, :].T), 4),
            "wo": chunked(np.ascontiguousarray(W_o[:, gf].T), 2),
            "ident": ident,
        })
    return in_maps


def kernel(x, W_q, W_k, W_v, W_o):
    from concourse.bass_utils import run_bass_kernel_spmd

    nc = get_nc()
    in_maps = make_in_maps(x, W_q, W_k, W_v, W_o)
    res = run_bass_kernel_spmd(nc, in_maps, list(range(8)))
    parts = [res.results[c]["out"] for c in range(8)]
    out = np.stack([parts[2 * b] + parts[2 * b + 1] for b in range(B)])
    return np.ascontiguousarray(out.astype(np.float32))
